# revision 14
# baseline (speedup 1.0000x reference)
"""Trainium2 Bass kernel for nn_Memory_73701638800014 (scatter_memory).

Contract: kernel(**inputs) takes FULL unsharded numpy inputs (as in
reference.setup_inputs()) and returns the FULL [B, H] output.

Strategy (8 NeuronCores, SPMD single program):
  - memory [50000, 8, 128] row-sharded: core k owns regions
    [k*6250, (k+1)*6250)  (25.6 MB/core in HBM; only indexed rows are read).
  - batch items partitioned by owning core (data parallel on reads),
    padded per-core to N_CAP (multiple of 128). Host permutes o_emb_r
    rows to match and inverse-permutes the outputs.
  - tiny params (attn_W, sim_w, forget_w, o_emb_w, memory[o_rg] row)
    replicated to all cores.
  - write phase (gather o_rg row -> sigmoid forget gate -> new slot) is
    computed on device on every core; the new slot is scattered to a
    scratch DRAM row, and a bounds-checked indirect gather patches the
    (rare) batch items whose region == o_rg.

Math (exact reassociation of the reference):
  w1' = attn_W @ sim_w[:H];  w2' = attn_W @ sim_w[H:]
  t1[b]   = o_emb_r[b] . w1'
  t2[b,s] = km[b,s] . w2'
  score   = softmax_s(relu(t1 + t2 + sim_b))
  out[b]  = (sum_s score[b,s] * km[b,s]) @ attn_W
"""

import os
import sys

import numpy as np

sys.path.insert(0, "/opt/trn_rl_repo")

R, S, H = 50000, 8, 128
B = 4096
NCORES = 8
RSHARD = R // NCORES  # 6250
SH = S * H  # 1024
P = 128

LAST_RESULT = None  # BassKernelResults of the most recent run (for profiling)


def _build_bass(n_tiles: int, b0: float):
    """SPMD Bass program for one core; N_CAP = n_tiles * 128 items.

    Tiles are processed in supersteps of 2 (big fused ops across 256 items)
    to cut instruction count and cross-engine sync overhead.
    """
    import concourse.bacc as bacc
    import concourse.bass as bass
    import concourse.mybir as mybir
    from concourse.masks import make_identity
    from concourse.tile import TileContext

    f32 = mybir.dt.float32
    bf16 = mybir.dt.bfloat16
    i32 = mybir.dt.int32
    AF = mybir.ActivationFunctionType
    ALU = mybir.AluOpType

    nc = bacc.Bacc("TRN2", target_bir_lowering=False, debug=True)

    # ---- DRAM I/O ----
    mem = nc.dram_tensor("mem", [RSHARD + 1, SH], f32, kind="ExternalInput")
    idx_all = nc.dram_tensor("idx", [P, n_tiles], i32, kind="ExternalInput")
    oeT = nc.dram_tensor("oeT", [P, n_tiles * P], f32, kind="ExternalInput")
    # params128: [128, 258] = attn_W | attn_W.T | sim_w as two cols
    p128 = nc.dram_tensor("p128", [P, 2 * P + 2], f32, kind="ExternalInput")
    # params8: [8, 512] = wrow | o_emb_w bcast | (-fw1) bcast | fw2 bcast
    p8 = nc.dram_tensor("p8", [S, 4 * P], f32, kind="ExternalInput")
    out = nc.dram_tensor("out", [n_tiles * P, P], f32, kind="ExternalOutput")

    groups = []
    t = 0
    while t < n_tiles:
        groups.append(list(range(t, min(t + 2, n_tiles))))
        t += 2

    with TileContext(nc) as tc:
        with (
            tc.tile_pool(name="const", bufs=1) as cpool,
            tc.tile_pool(name="work", bufs=2) as wpool,
            tc.tile_pool(name="small", bufs=2) as spool,
            tc.tile_pool(name="psum", bufs=2, space="PSUM") as ppool,
            tc.tile_pool(name="psmall", bufs=1, space="PSUM") as pspool,
        ):
            # ---- load constants ----
            params = cpool.tile([P, 2 * P + 2], f32)
            nc.sync.dma_start(out=params[:], in_=p128[:])
            attn_w = params[:, 0:P]
            attn_wT = params[:, P : 2 * P]
            sw12 = params[:, 2 * P : 2 * P + 2]

            par8 = cpool.tile([S, 4 * P], f32)
            nc.sync.dma_start(out=par8[:], in_=p8[:])
            wrow = par8[:, 0:P]
            oewr = par8[:, P : 2 * P]
            fw1rn = par8[:, 2 * P : 3 * P]  # negated on host
            fw2r = par8[:, 3 * P : 4 * P]

            idxs = cpool.tile([P, n_tiles], i32)
            nc.sync.dma_start(out=idxs[:], in_=idx_all[:])

            oet = cpool.tile([P, n_tiles * P], f32)
            nc.sync.dma_start(out=oet[:], in_=oeT[:])

            ident = cpool.tile([P, P], bf16)
            make_identity(nc, ident[:])

            attn_wb = cpool.tile([P, P], bf16)
            nc.vector.tensor_copy(out=attn_wb[:], in_=attn_w)

            ones_row = cpool.tile([1, P], f32)
            nc.gpsimd.memset(ones_row[:], 1.0)

            b0t = cpool.tile([P, 1], f32)
            nc.gpsimd.memset(b0t[:], b0)

            # ---- derived params ----
            # w1col [128, 1] = attn_W @ sim_w[:H]
            w12c_ps = pspool.tile([P, 2], f32, space="PSUM", tag="setup")
            nc.tensor.matmul(out=w12c_ps[:], lhsT=attn_wT, rhs=sw12, start=True, stop=True)
            w12col = cpool.tile([P, 2], f32)
            nc.vector.tensor_copy(out=w12col[:], in_=w12c_ps[:])
            # w2row [1, 128] = (attn_W @ sim_w[H:]).T
            w2r_ps = pspool.tile([1, P], f32, space="PSUM", tag="setup")
            nc.tensor.matmul(out=w2r_ps[:], lhsT=sw12[:, 1:2], rhs=attn_wT, start=True, stop=True)
            w2row = cpool.tile([1, P], f32)
            nc.vector.tensor_copy(out=w2row[:], in_=w2r_ps[:])
            # w2rep [128, 128] bf16: w2' broadcast across partitions
            w2rep_ps = pspool.tile([P, P], f32, space="PSUM", tag="setup")
            nc.tensor.matmul(out=w2rep_ps[:], lhsT=ones_row[:], rhs=w2row[:], start=True, stop=True)
            w2rep = cpool.tile([P, P], bf16)
            nc.vector.tensor_copy(out=w2rep[:], in_=w2rep_ps[:])

            # ---- t1 for all tiles: t1b_all[:, t] = oe_t . w1' + b0 ----
            t1_ps = pspool.tile([P, n_tiles], f32, space="PSUM", tag="t1ps")
            for t in range(n_tiles):
                nc.tensor.matmul(
                    out=t1_ps[:, t : t + 1],
                    lhsT=oet[:, t * P : (t + 1) * P],
                    rhs=w12col[:, 0:1],
                    start=True, stop=True,
                )
            t1b = cpool.tile([P, n_tiles], f32)
            nc.scalar.activation(out=t1b[:], in_=t1_ps[:], func=AF.Copy, scale=1.0)
            t1bb = cpool.tile([P, n_tiles], f32)
            nc.vector.tensor_scalar_add(out=t1bb[:], in0=t1b[:], scalar1=b0t[:, 0:1])

            # ---- write phase: new_slot from wrow (sigmoid via exp) ----
            c0n = spool.tile([S, 1], f32)
            junk8 = spool.tile([S, P], f32)
            nc.vector.tensor_tensor(out=junk8[:], in0=oewr, in1=fw1rn, op=ALU.mult)
            nc.vector.reduce_sum(out=c0n[:], in_=junk8[:], axis=mybir.AxisListType.X)
            dotc = spool.tile([S, 1], f32)
            junk8b = spool.tile([S, P], f32)
            nc.vector.tensor_tensor(out=junk8b[:], in0=wrow, in1=fw2r, op=ALU.mult)
            nc.vector.reduce_sum(out=dotc[:], in_=junk8b[:], axis=mybir.AxisListType.X)
            # gate = 1 / (1 + exp(-(dot + c0)))
            en = spool.tile([S, 1], f32)
            nc.scalar.activation(out=en[:], in_=dotc[:], func=AF.Exp, bias=c0n[:, 0:1], scale=-1.0)
            den = spool.tile([S, 1], f32)
            nc.vector.tensor_scalar_add(out=den[:], in0=en[:], scalar1=1.0)
            gate = spool.tile([S, 1], f32)
            nc.vector.reciprocal(out=gate[:], in_=den[:])
            delta = spool.tile([S, P], f32)
            nc.vector.tensor_tensor(out=delta[:], in0=oewr, in1=wrow, op=ALU.subtract)
            wdelta = spool.tile([S, P], f32)
            nc.vector.tensor_scalar_mul(out=wdelta[:], in0=delta[:], scalar1=gate[:, 0:1])
            new_slot = spool.tile([S, P], f32)
            nc.vector.tensor_tensor(out=new_slot[:], in0=wrow, in1=wdelta[:], op=ALU.add)
            # scatter new_slot into the shard scratch row (read by hit items)
            nc.sync.dma_start(
                out=mem[RSHARD, :].rearrange("(s h) -> s h", s=S), in_=new_slot[:]
            )

            out_sb = cpool.tile([P, n_tiles * P], f32)

            # ---- main loop over supersteps ----
            for gi, gts in enumerate(groups):
                g = len(gts)
                t0 = gts[0]
                km = wpool.tile([P, 2 * SH], bf16, tag="km")
                for j, t in enumerate(gts):
                    src_ap = mem[:, :] if t == 0 else mem[0:RSHARD, :]
                    nc.gpsimd.indirect_dma_start(
                        out=km[:, j * SH : (j + 1) * SH],
                        out_offset=None,
                        in_=src_ap,
                        in_offset=bass.IndirectOffsetOnAxis(ap=idxs[:, t : t + 1], axis=0),
                    )
                GS = g * S  # slot groups in this superstep
                km3 = km[:, : g * SH].rearrange("p (q h) -> p q h", q=GS)

                # t2 = per-slot dot with w2'
                tmp = wpool.tile([P, 2 * SH], bf16, tag="tmp")
                tmp3 = tmp[:, : g * SH].rearrange("p (q h) -> p q h", q=GS)
                w2b = w2rep[:].rearrange("p (o h) -> p o h", o=1).broadcast_to([P, GS, P])
                nc.vector.tensor_tensor(out=tmp3, in0=km3, in1=w2b, op=ALU.mult)
                t2g = spool.tile([P, 2 * S], f32, tag="t2g")
                nc.vector.reduce_sum(out=t2g[:, :GS], in_=tmp3, axis=mybir.AxisListType.X)

                # logits = relu(t2 + t1 + b0); softmax over s within each tile
                lg = spool.tile([P, 2 * S], f32, tag="lg")
                t1v = t1bb[:, t0 : t0 + g].rearrange("p (g o) -> p g o", o=1).broadcast_to([P, g, S])
                nc.vector.tensor_tensor(
                    out=lg[:, :GS].rearrange("p (g s) -> p g s", g=g),
                    in0=t2g[:, :GS].rearrange("p (g s) -> p g s", g=g),
                    in1=t1v, op=ALU.add,
                )
                rl = spool.tile([P, 2 * S], f32, tag="rl")
                nc.scalar.activation(out=rl[:, :GS], in_=lg[:, :GS], func=AF.Relu)
                ex = spool.tile([P, 2 * S], f32, tag="ex")
                nc.scalar.activation(out=ex[:, :GS], in_=rl[:, :GS], func=AF.Exp)
                se = spool.tile([P, 2], f32, tag="se")
                nc.vector.reduce_sum(
                    out=se[:, :g],
                    in_=ex[:, :GS].rearrange("p (g s) -> p g s", g=g),
                    axis=mybir.AxisListType.X,
                )
                rs = spool.tile([P, 2], f32, tag="rs")
                nc.vector.reciprocal(out=rs[:, :g], in_=se[:, :g])
                score = spool.tile([P, 2 * S], bf16, tag="score")
                rsv = rs[:, :g].rearrange("p (g o) -> p g o", o=1).broadcast_to([P, g, S])
                nc.vector.tensor_tensor(
                    out=score[:, :GS].rearrange("p (g s) -> p g s", g=g),
                    in0=ex[:, :GS].rearrange("p (g s) -> p g s", g=g),
                    in1=rsv, op=ALU.mult,
                )

                # wkm = km * score (broadcast over h); split DVE / GPSIMD
                wkm = wpool.tile([P, 2 * SH], bf16, tag="wkm")
                wkm3 = wkm[:, : g * SH].rearrange("p (q h) -> p q h", q=GS)
                sc3 = score[:, :GS].rearrange("p (q o) -> p q o", o=1).broadcast_to([P, GS, P])
                hq = GS // 2 if g == 2 else GS
                nc.gpsimd.tensor_tensor(
                    out=wkm3[:, :hq, :], in0=km3[:, :hq, :], in1=sc3[:, :hq, :], op=ALU.mult
                )
                if g == 2:
                    nc.vector.tensor_tensor(
                        out=wkm3[:, hq:, :], in0=km3[:, hq:, :], in1=sc3[:, hq:, :], op=ALU.mult
                    )

                # ctx[i, t, h] = sum_s wkm[i, t, s, h]: 3-level contiguous tree
                w4 = wkm[:, : g * SH].rearrange("p (g s h) -> p g s h", g=g, s=S)
                red4 = wpool.tile([P, SH], bf16, tag="red4")  # [P, g, 4, 128]
                r4 = red4[:, : g * SH // 2].rearrange("p (g s h) -> p g s h", g=g, s=S // 2)
                nc.gpsimd.tensor_tensor(out=r4, in0=w4[:, :, 0:4, :], in1=w4[:, :, 4:8, :], op=ALU.add)
                red2 = wpool.tile([P, SH // 2], bf16, tag="red2")  # [P, g, 2, 128]
                r2 = red2[:, : g * SH // 4].rearrange("p (g s h) -> p g s h", g=g, s=S // 4)
                nc.vector.tensor_tensor(out=r2, in0=r4[:, :, 0:2, :], in1=r4[:, :, 2:4, :], op=ALU.add)
                ctx = wpool.tile([P, SH // 4], bf16, tag="ctx")  # [P, g, 128]
                cx = ctx[:, : g * SH // 8].rearrange("p (g o h) -> p g o h", g=g, o=1)
                nc.vector.tensor_tensor(out=cx, in0=r2[:, :, 0:1, :], in1=r2[:, :, 1:2, :], op=ALU.add)

                # per tile: transpose ctx, project with attn_W
                for j, t in enumerate(gts):
                    ctxT_ps = ppool.tile([P, P], bf16, space="PSUM", tag="ctxT")
                    nc.tensor.transpose(
                        out=ctxT_ps[:], in_=ctx[:, j * P : (j + 1) * P], identity=ident[:]
                    )
                    ctxT = wpool.tile([P, P], bf16, tag="ctxTsb")
                    nc.scalar.copy(out=ctxT[:], in_=ctxT_ps[:])
                    mo_ps = ppool.tile([P, P], f32, space="PSUM", tag="mo")
                    nc.tensor.matmul(out=mo_ps[:], lhsT=ctxT[:], rhs=attn_wb[:], start=True, stop=True)
                    if j % 2 == 0:
                        nc.vector.tensor_copy(out=out_sb[:, t * P : (t + 1) * P], in_=mo_ps[:])
                    else:
                        nc.scalar.copy(out=out_sb[:, t * P : (t + 1) * P], in_=mo_ps[:])

            # ---- one output DMA ----
            nc.sync.dma_start(
                out=out[:, :].rearrange("(t i) h -> i t h", i=P),
                in_=out_sb[:].rearrange("p (t h) -> p t h", t=n_tiles),
            )

    nc.compile()
    return nc


def _install_ntff_hook():
    """Provide antenv.axon_hooks (NTFF profiling) if the image lacks it."""
    import types

    try:
        from antenv.axon_hooks import get_axon_ntff_profile_hook  # noqa: F401

        return
    except ImportError:
        pass
    import contextlib
    import ctypes

    import antenv

    so_path = "/opt/axon/libaxon_pjrt.so"

    def _make_hook():
        try:
            lib = ctypes.CDLL(so_path)
        except OSError:
            return None
        if not hasattr(lib, "axon_start_nrt_profile"):
            return None
        lib.axon_start_nrt_profile.argtypes = [
            ctypes.POINTER(ctypes.c_int64),
            ctypes.c_size_t,
        ]
        lib.axon_start_nrt_profile.restype = ctypes.c_int64
        lib.axon_stop_nrt_profile.argtypes = [ctypes.c_char_p]
        lib.axon_stop_nrt_profile.restype = ctypes.c_int64

        @contextlib.contextmanager
        def _hook(output_dir, device_ids):
            import jax

            jax.devices()
            if device_ids:
                ids = (ctypes.c_int64 * len(device_ids))(*device_ids)
                rc = lib.axon_start_nrt_profile(ids, len(device_ids))
            else:
                rc = lib.axon_start_nrt_profile(None, 0)
            if rc != 0:
                raise RuntimeError(f"axon_start_nrt_profile rc={rc}")
            try:
                yield
            finally:
                n = lib.axon_stop_nrt_profile(str(output_dir).encode())
                print(f"ntff profile: {n} file(s) -> {output_dir}", file=sys.stderr)

        return _hook

    mod = types.ModuleType("antenv.axon_hooks")
    _hook_obj = _make_hook()
    mod.get_axon_ntff_profile_hook = lambda: _hook_obj
    mod.set_axon_ntff_profile_hook = lambda h: None
    sys.modules["antenv.axon_hooks"] = mod
    antenv.axon_hooks = mod


def kernel(**inputs) -> np.ndarray:
    global LAST_RESULT
    _install_ntff_hook()
    from concourse.bass_utils import run_bass_kernel_spmd

    memory = np.ascontiguousarray(np.asarray(inputs["memory"], dtype=np.float32))
    o_emb_w = np.asarray(inputs["o_emb_w"], dtype=np.float32)
    o_emb_r = np.ascontiguousarray(np.asarray(inputs["o_emb_r"], dtype=np.float32))
    attn_W = np.ascontiguousarray(np.asarray(inputs["attn_W"], dtype=np.float32))
    sim_w = np.asarray(inputs["sim_w"], dtype=np.float32)
    sim_b = np.asarray(inputs["sim_b"], dtype=np.float32)
    forget_w = np.asarray(inputs["forget_w"], dtype=np.float32)
    o_rg = int(np.asarray(inputs["o_rg"]))
    d_rg = np.asarray(inputs["d_rg"]).astype(np.int64)

    mem2d = memory.reshape(R, SH)
    owner_of_w = o_rg // RSHARD

    # --- partition batch by owning core; hits (d_rg == o_rg) first ---
    owner = d_rg // RSHARD
    locs, nks = [], []
    for k in range(NCORES):
        lk = np.where(owner == k)[0]
        if k == owner_of_w:
            hits = lk[d_rg[lk] == o_rg]
            nonhits = lk[d_rg[lk] != o_rg]
            assert len(hits) <= P, "too many batch items hit the written region"
            lk = np.concatenate([hits, nonhits])
        locs.append(lk)
        nks.append(len(lk))
    n_cap = max(P, int(np.ceil(max(nks) / P) * P))
    n_tiles = n_cap // P

    b0 = float(sim_b.reshape(-1)[0])
    nc = _build_bass(n_tiles, b0)

    # --- shared (replicated) small params ---
    fw1 = -forget_w[:H, 0]  # negated: device computes -c0 directly
    fw2 = forget_w[H:, 0]
    sw12 = np.stack([sim_w[:H, 0], sim_w[H:, 0]], axis=1)  # [128, 2]
    p128 = np.concatenate([attn_W, attn_W.T, sw12], axis=1).astype(np.float32)
    wrow = mem2d[o_rg].reshape(S, H)
    p8 = np.concatenate(
        [wrow, np.tile(o_emb_w, (S, 1)), np.tile(fw1, (S, 1)), np.tile(fw2, (S, 1))],
        axis=1,
    ).astype(np.float32)
    p128 = np.ascontiguousarray(p128)
    p8 = np.ascontiguousarray(p8)

    in_maps = []
    for k in range(NCORES):
        lk, nk = locs[k], nks[k]
        li = (d_rg[lk] - k * RSHARD).astype(np.int32)
        li[d_rg[lk] == o_rg] = RSHARD  # hit items read the scratch row
        li_pad = np.zeros(n_cap, dtype=np.int32)
        li_pad[:nk] = li
        # idx layout [128, n_tiles]: col t = tile t's local indices
        idx_cols = np.ascontiguousarray(li_pad.reshape(n_tiles, P).T)

        oe = np.zeros((n_cap, H), dtype=np.float32)
        oe[:nk] = o_emb_r[lk]
        oeT = np.ascontiguousarray(oe.T)

        in_maps.append(
            {
                "mem": np.concatenate(
                    [mem2d[k * RSHARD : (k + 1) * RSHARD], np.zeros((1, SH), np.float32)]
                ),
                "idx": idx_cols,
                "oeT": oeT,
                "p128": p128,
                "p8": p8,
            }
        )

    trace = os.environ.get("KERNEL_TRACE", "0") == "1"
    res = run_bass_kernel_spmd(nc, in_maps, list(range(NCORES)), trace=trace)
    LAST_RESULT = res

    full = np.zeros((B, H), dtype=np.float32)
    for k in range(NCORES):
        if nks[k] > 0:
            full[locs[k]] = res.results[k]["out"][: nks[k]]
    return full
